# revision 1
# baseline (speedup 1.0000x reference)
"""DeepseekV2 MLA decode (matrix-absorbed) on 8 Trainium2 NeuronCores.

Sharding:
  - W_DQ row-sharded (contraction) -> partial cQ -> AllReduce (49KB) -> RMSNorm
    computed redundantly on every core (ln_w is folded into W_QR/W_UQ_UK host-side).
  - W_QR / W_UQ_UK head-sharded (16 of 128 heads per core).
  - AllGather of q (per-core [8,16,576] -> [8,8,16,576]).
  - Attention sharded over kv_len (1024 of 8192 positions per core, all 128 heads);
    rope applied to k with *relative* positions (q stays un-roped: R(a)q . R(b)k =
    q . R(b-a)k), softmax exp without max subtraction (scores are O(6)), partial
    exp-sums via activation accum_out; partial (attn, lsum) -> ReduceScatter(add)
    grouped by head-block so core c receives exactly its 16 heads.
  - W_UV_O row-sharded (same 16 heads); fp32r matmuls with the big weight as the
    moving operand; final AllReduce of [8,5120] partial outputs.
"""
import sys

if "/opt/trn_rl_repo" not in sys.path:
    sys.path.insert(0, "/opt/trn_rl_repo")

import numpy as np

N_CORES = 8
B = 8           # batch
H = 5120        # hidden
NH = 128        # heads
QLR = 1536      # q lora rank
ROPE = 64
KVLR = 512
KV = 8192
THETA = 10000.0
SCALE = 192.0 ** -0.5

HL = NH // N_CORES      # 16 local heads
KVL = KV // N_CORES     # 1024 local kv positions
HD = H // N_CORES       # 640 local hidden (stage-1 contraction shard)
KT = KVL // 128         # 8 kv tiles of 128 per core

_CACHE = {}


def build_nc(sim=False):
    import concourse.bacc as bacc
    import concourse.mybir as mybir
    import concourse.tile as tile

    F32 = mybir.dt.float32
    F32R = mybir.dt.float32r
    AF = mybir.ActivationFunctionType

    nc = bacc.Bacc("TRN2", target_bir_lowering=False, debug=False,
                   num_devices=(1 if sim else N_CORES))

    # ---- per-core inputs ----
    hs = nc.dram_tensor("hs", [B, HD], F32R, kind="ExternalInput")
    wdq = nc.dram_tensor("wdq", [HD, QLR], F32R, kind="ExternalInput")
    wqr = nc.dram_tensor("wqr", [QLR, HL * ROPE], F32R, kind="ExternalInput")
    wuk = nc.dram_tensor("wuk", [QLR, HL * KVLR], F32R, kind="ExternalInput")
    ckv = nc.dram_tensor("ckv", [B, KVL, KVLR], F32R, kind="ExternalInput")
    kpe = nc.dram_tensor("kpe", [B, KVL, ROPE], F32, kind="ExternalInput")
    cost = nc.dram_tensor("cost", [KVL, ROPE // 2], F32, kind="ExternalInput")
    sint = nc.dram_tensor("sint", [KVL, ROPE // 2], F32, kind="ExternalInput")
    ident = nc.dram_tensor("ident", [128, 128], F32R, kind="ExternalInput")
    wuvo = nc.dram_tensor("wuvo", [HL * KVLR, H], F32R, kind="ExternalInput")
    out = nc.dram_tensor("out", [B, H], F32, kind="ExternalOutput")

    RG = [list(range(N_CORES))]

    def coll(kind, op, in_t, out_t):
        if not sim:
            nc.gpsimd.collective_compute(kind, op, replica_groups=RG,
                                         ins=[in_t.opt()], outs=[out_t.opt()])
        elif kind == "AllGather":
            nc.sync.dma_start(out_t[0], in_t[:])
        elif kind == "ReduceScatter":
            nc.sync.dma_start(out_t[:], in_t[0])
        else:
            nc.sync.dma_start(out_t[:], in_t[:])

    with tile.TileContext(nc) as tc:
        with (
            tc.tile_pool(name="const", bufs=1) as cpool,
            tc.tile_pool(name="dram", bufs=1, space="DRAM") as dram,
            tc.tile_pool(name="wuvo_sb", bufs=6) as wvp,
            tc.tile_pool(name="tpack", bufs=2, space="PSUM") as tpp,
            tc.tile_pool(name="misc", bufs=1) as misc,
        ):
            idt = cpool.tile([128, 128], F32R)
            nc.sync.dma_start(idt[:], ident[:])
            eps = cpool.tile([8, 1], F32)
            nc.vector.memset(eps[:], 1e-6)
            # relative-position rope tables, laid out [p=128, t=KT, 32]
            ct_sb = cpool.tile([128, KT, ROPE // 2], F32)
            st_sb = cpool.tile([128, KT, ROPE // 2], F32)
            nc.sync.dma_start(ct_sb[:], cost[:].rearrange("(t p) i -> p t i", p=128))
            nc.sync.dma_start(st_sb[:], sint[:].rearrange("(t p) i -> p t i", p=128))

            # collective bounce buffers
            cq_ar_in = dram.tile([B, QLR], F32)
            cq_ar_out = dram.tile([B, QLR], F32)
            q_ag_in = dram.tile([B, HL, KVLR + ROPE], F32)
            q_ag_out = dram.tile([N_CORES, B, HL, KVLR + ROPE], F32)
            at_rs_in = dram.tile([N_CORES, B, HL, KVLR + 1], F32)
            at_rs_out = dram.tile([B, HL, KVLR + 1], F32)
            o_ar_in = dram.tile([B, H], F32)
            o_ar_out = dram.tile([B, H], F32)

            # =========== Stage 1: cQ = rmsnorm(hs @ W_DQ) ===========
            with (
                tc.tile_pool(name="s1", bufs=1) as s1,
                tc.tile_pool(name="s12ps", bufs=1, space="PSUM") as s1ps,
                tc.tile_pool(name="qnps", bufs=2, space="PSUM") as qnps,
                tc.tile_pool(name="wuk_sb", bufs=24) as wkp,
            ):
                hs_sb = s1.tile([B, HD], F32R)
                nc.sync.dma_start(hs_sb[:], hs[:])
                wdq_sb = s1.tile([128, 5, QLR], F32R)
                nc.sync.dma_start(wdq_sb[:], wdq[:].rearrange("(k p) j -> p k j", p=128))
                hsT = s1.tile([128, 5, 8], F32R)
                for k in range(5):
                    tp = tpp.tile([128, 8], F32R, tag="tp")
                    nc.tensor.transpose(tp[:], hs_sb[:, k * 128:(k + 1) * 128], idt[0:8, 0:8])
                    nc.vector.tensor_copy(hsT[:, k, :], tp[:])
                cq_ps = s1ps.tile([8, QLR], F32)
                for n in range(3):
                    for k in range(5):
                        nc.tensor.matmul(
                            cq_ps[:, n * 512:(n + 1) * 512],
                            hsT[:, k, :],
                            wdq_sb[:, k, n * 512:(n + 1) * 512],
                            start=(k == 0), stop=(k == 4),
                        )
                cqraw = s1.tile([8, QLR], F32)
                nc.scalar.copy(cqraw[:], cq_ps[:])
                nc.sync.dma_start(cq_ar_in[:], cqraw[:])
                coll("AllReduce", mybir.AluOpType.add, cq_ar_in, cq_ar_out)
                cqsum = s1.tile([8, QLR], F32)
                nc.sync.dma_start(cqsum[:], cq_ar_out[:])
                # rmsnorm (ln_w folded into the weights host-side)
                sq = s1.tile([8, QLR], F32)
                ssq = s1.tile([8, 1], F32)
                nc.scalar.activation(sq[:], cqsum[:], AF.Square, accum_out=ssq[:])
                sdev = s1.tile([8, 1], F32)
                nc.scalar.activation(sdev[:], ssq[:], AF.Sqrt, bias=eps[:], scale=1.0 / QLR)
                rinv = s1.tile([8, 1], F32)
                nc.vector.reciprocal(rinv[:], sdev[:])
                cqn = s1.tile([8, QLR], F32R)
                nc.vector.tensor_scalar_mul(cqn[:], cqsum[:], rinv[:])
                cqnT = s1.tile([128, 12, 8], F32R)
                for k in range(12):
                    tp = tpp.tile([128, 8], F32R, tag="tp")
                    nc.tensor.transpose(tp[:], cqn[:, k * 128:(k + 1) * 128], idt[0:8, 0:8])
                    nc.vector.tensor_copy(cqnT[:, k, :], tp[:])

                # =========== Stage 2: q projections for 16 local heads ===========
                qpe_sb = s1.tile([8, HL * ROPE], F32)
                for n in range(2):
                    ps_q = qnps.tile([8, 512], F32)
                    for k in range(12):
                        wt = wkp.tile([128, 512], F32R, tag="wuk")
                        nc.sync.dma_start(
                            wt[:], wqr[:].rearrange("(kk p) n -> p kk n", p=128)[:, k, n * 512:(n + 1) * 512]
                        )
                        nc.tensor.matmul(ps_q[:], cqnT[:, k, :], wt[:],
                                         start=(k == 0), stop=(k == 11))
                    nc.scalar.copy(qpe_sb[:, n * 512:(n + 1) * 512], ps_q[:])
                qn_sb = s1.tile([8, HL * KVLR], F32)
                for n in range(16):
                    ps_q = qnps.tile([8, 512], F32)
                    for k in range(12):
                        wt = wkp.tile([128, 512], F32R, tag="wuk")
                        nc.sync.dma_start(
                            wt[:], wuk[:].rearrange("(kk p) n -> p kk n", p=128)[:, k, n * 512:(n + 1) * 512]
                        )
                        nc.tensor.matmul(ps_q[:], cqnT[:, k, :], wt[:],
                                         start=(k == 0), stop=(k == 11))
                    nc.scalar.copy(qn_sb[:, n * 512:(n + 1) * 512], ps_q[:])
                # deinterleave q_pe (concat-halves permutation, matching k below)
                qpe2 = s1.tile([8, HL, ROPE], F32)
                qv = qpe_sb[:].rearrange("b (h r) -> b h r", h=HL)
                nc.vector.tensor_copy(qpe2[:, :, 0:32], qv[:, :, 0:ROPE:2])
                nc.vector.tensor_copy(qpe2[:, :, 32:64], qv[:, :, 1:ROPE:2])
                # pack q into the allgather buffer
                nc.sync.dma_start(
                    q_ag_in[:, :, 0:KVLR],
                    qn_sb[:].rearrange("b (h l) -> b h l", h=HL),
                )
                nc.sync.dma_start(q_ag_in[:, :, KVLR:KVLR + ROPE], qpe2[:])
                coll("AllGather", mybir.AluOpType.bypass, q_ag_in, q_ag_out)

            # =========== Stage 3: attention over local kv shard, all 128 heads ===========
            with (
                tc.tile_pool(name="s3", bufs=2) as s3,
                tc.tile_pool(name="s3o", bufs=2) as s3o,
                tc.tile_pool(name="scps", bufs=2, space="PSUM") as scps,
                tc.tile_pool(name="atps", bufs=2, space="PSUM") as atps,
            ):
                for b in range(B):
                    qn_all = s3.tile([128, KVLR], F32R, tag="qn_all")
                    nc.sync.dma_start(qn_all[:], q_ag_out[:, b, :, 0:KVLR].bitcast(F32R))
                    qe_all = s3.tile([128, ROPE], F32R, tag="qe_all")
                    nc.sync.dma_start(qe_all[:], q_ag_out[:, b, :, KVLR:KVLR + ROPE].bitcast(F32R))
                    # transpose q
                    qnT = s3.tile([128, 4, 128], F32R, tag="qnT")
                    tp = tpp.tile([128, 512], F32R, tag="tp")
                    for lc in range(4):
                        nc.tensor.transpose(tp[:, lc * 128:(lc + 1) * 128],
                                            qn_all[:, lc * 128:(lc + 1) * 128], idt[:])
                    nc.vector.tensor_copy(qnT[:].rearrange("p a b -> p (a b)"), tp[:])
                    qeT = s3.tile([64, 128], F32R, tag="qeT")
                    tpq = tpp.tile([64, 128], F32R, tag="tp")
                    nc.tensor.transpose(tpq[:], qe_all[:], idt[:])
                    nc.vector.tensor_copy(qeT[:], tpq[:])
                    # load ckv tile [128, t, l]
                    ckv_sb = s3.tile([128, KT, KVLR], F32R, tag="ckv")
                    nc.sync.dma_start(ckv_sb[:], ckv[b].rearrange("(t p) l -> p t l", p=128))
                    # transpose ckv -> [l=4x128, kv=KVL]
                    ckvT = s3.tile([128, 4, KVL], F32R, tag="ckvT")
                    for lc in range(4):
                        for g in range(KT // 4):
                            tp = tpp.tile([128, 512], F32R, tag="tp")
                            for j in range(4):
                                t = g * 4 + j
                                nc.tensor.transpose(tp[:, j * 128:(j + 1) * 128],
                                                    ckv_sb[:, t, lc * 128:(lc + 1) * 128], idt[:])
                            nc.vector.tensor_copy(ckvT[:, lc, g * 512:(g + 1) * 512], tp[:])
                    # rope(k) with relative positions, deinterleaved into halves
                    kpe_sb = s3.tile([128, KT, ROPE], F32, tag="kpe")
                    nc.sync.dma_start(kpe_sb[:], kpe[b].rearrange("(t p) r -> p t r", p=128))
                    k0c = s3.tile([128, KT, 32], F32, tag="k0c")
                    k1s = s3.tile([128, KT, 32], F32, tag="k1s")
                    k0s = s3.tile([128, KT, 32], F32, tag="k0s")
                    k1c = s3.tile([128, KT, 32], F32, tag="k1c")
                    ev = kpe_sb[:, :, 0:ROPE:2]
                    od = kpe_sb[:, :, 1:ROPE:2]
                    nc.vector.tensor_mul(k0c[:], ev, ct_sb[:])
                    nc.vector.tensor_mul(k1s[:], od, st_sb[:])
                    nc.vector.tensor_mul(k0s[:], ev, st_sb[:])
                    nc.vector.tensor_mul(k1c[:], od, ct_sb[:])
                    ke = s3.tile([128, KT, ROPE], F32R, tag="ke")
                    nc.vector.tensor_sub(ke[:, :, 0:32], k0c[:], k1s[:])
                    nc.vector.tensor_add(ke[:, :, 32:64], k0s[:], k1c[:])
                    # transpose ke -> [64, kv]
                    keT = s3.tile([64, KVL], F32R, tag="keT")
                    for g in range(KT // 4):
                        tp = tpp.tile([64, 512], F32R, tag="tp")
                        for j in range(4):
                            t = g * 4 + j
                            nc.tensor.transpose(tp[:, j * 128:(j + 1) * 128], ke[:, t, :], idt[:])
                        nc.vector.tensor_copy(keT[:, g * 512:(g + 1) * 512], tp[:])
                    # scores = qn . ckv^T + qe . ke^T   [128h, KVL]
                    sc_ps = scps.tile([128, KVL], F32)
                    for t2 in range(KVL // 512):
                        sl = slice(t2 * 512, (t2 + 1) * 512)
                        for lc in range(4):
                            nc.tensor.matmul(sc_ps[:, sl], qnT[:, lc, :], ckvT[:, lc, sl],
                                             start=(lc == 0), stop=False)
                        nc.tensor.matmul(sc_ps[:, sl], qeT[:], keT[:, sl],
                                         start=False, stop=True)
                    # probs (unnormalized) + partial lsum
                    attn_sb = s3o.tile([128, KVLR + 1], F32, tag="attn")
                    probs = s3.tile([128, KVL], F32R, tag="probs")
                    nc.scalar.activation(probs[:], sc_ps[:], AF.Exp, scale=SCALE,
                                         accum_out=attn_sb[:, KVLR:KVLR + 1])
                    # probs^T
                    probsT = s3.tile([128, KT, 128], F32R, tag="probsT")
                    for g in range(KT // 4):
                        tp = tpp.tile([128, 512], F32R, tag="tp")
                        for j in range(4):
                            t = g * 4 + j
                            nc.tensor.transpose(tp[:, j * 128:(j + 1) * 128],
                                                probs[:, t * 128:(t + 1) * 128], idt[:])
                        nc.vector.tensor_copy(
                            probsT[:, g * 4:(g + 1) * 4, :].rearrange("p a b -> p (a b)"), tp[:])
                    # attn partial = probs^T . ckv  [128h, KVLR]
                    at_ps = atps.tile([128, KVLR], F32)
                    for t in range(KT):
                        nc.tensor.matmul(at_ps[:], probsT[:, t, :], ckv_sb[:, t, :],
                                         start=(t == 0), stop=(t == KT - 1))
                    nc.vector.tensor_copy(attn_sb[:, 0:KVLR], at_ps[:])
                    # scatter into reduce buffer grouped by head-block
                    for ci in range(N_CORES):
                        nc.sync.dma_start(at_rs_in[ci, b, :, :],
                                          attn_sb[ci * HL:(ci + 1) * HL, :])
                coll("ReduceScatter", mybir.AluOpType.add, at_rs_in, at_rs_out)

            # =========== Stage 4: out = (attn/lsum) @ W_UV_O, head shard ===========
            with (
                tc.tile_pool(name="s4", bufs=1) as s4,
                tc.tile_pool(name="oaps", bufs=1, space="PSUM") as oaps,
            ):
                o_sb = s4.tile([8, HL, KVLR + 1], F32)
                nc.sync.dma_start(o_sb[:], at_rs_out[:])
                linv = s4.tile([8, HL], F32)
                nc.vector.reciprocal(linv[:], o_sb[:, :, KVLR])
                osc = s4.tile([8, HL, KVLR], F32R)
                for h in range(HL):
                    nc.vector.tensor_scalar_mul(osc[:, h, :], o_sb[:, h, 0:KVLR],
                                                linv[:, h:h + 1])
                aT = s4.tile([128, HL * 4, 8], F32R)
                for h in range(HL):
                    tp = tpp.tile([128, 32], F32R, tag="tp")
                    for lc in range(4):
                        nc.tensor.transpose(tp[:, lc * 8:(lc + 1) * 8],
                                            osc[:, h, lc * 128:(lc + 1) * 128], idt[0:8, 0:8])
                    nc.vector.tensor_copy(
                        aT[:, h * 4:(h + 1) * 4, :].rearrange("p a b -> p (a b)"), tp[:])
                outp = s4.tile([8, H], F32)
                NHALF = H // 2
                for half in range(2):
                    o_ps = oaps.tile([8, NHALF], F32)
                    for r in range(64):
                        wt = wvp.tile([128, NHALF], F32R, tag="wuvo")
                        nc.sync.dma_start(
                            wt[:], wuvo[r * 128:(r + 1) * 128,
                                        half * NHALF:(half + 1) * NHALF])
                        for n5 in range(NHALF // 512):
                            nc.tensor.matmul(o_ps[:, n5 * 512:(n5 + 1) * 512],
                                             aT[:, r, :], wt[:, n5 * 512:(n5 + 1) * 512],
                                             start=(r == 0), stop=(r == 63))
                    nc.scalar.copy(outp[:, half * NHALF:(half + 1) * NHALF], o_ps[:])
                nc.sync.dma_start(o_ar_in[:], outp[:])
                coll("AllReduce", mybir.AluOpType.add, o_ar_in, o_ar_out)
                nc.sync.dma_start(out[:], o_ar_out[:])

    nc.compile()
    return nc


def make_in_maps(hidden_states, compressed_kv_normed_cache, k_pe_cache,
                 W_DQ, ln_w, W_QR, W_UQ_UK, W_UV_O):
    f32 = np.float32
    hidden_states = np.asarray(hidden_states, f32)
    ckv = np.asarray(compressed_kv_normed_cache, f32)
    kpe = np.asarray(k_pe_cache, f32)
    W_DQ = np.asarray(W_DQ, f32)
    ln_w = np.asarray(ln_w, f32)
    W_QR = np.asarray(W_QR, f32) * ln_w[:, None]
    W_UQ_UK = np.asarray(W_UQ_UK, f32) * ln_w[:, None]
    W_UV_O = np.asarray(W_UV_O, f32)

    inv = 1.0 / (THETA ** (np.arange(0, ROPE, 2, dtype=np.float64) / ROPE))
    rel = (np.arange(KV, dtype=np.float64) - (KV - 1))[:, None] * inv[None, :]
    cost = np.cos(rel).astype(f32)
    sint = np.sin(rel).astype(f32)
    ident = np.eye(128, dtype=f32)

    c = np.ascontiguousarray
    in_maps = []
    for ci in range(N_CORES):
        in_maps.append({
            "hs": c(hidden_states[:, ci * HD:(ci + 1) * HD]),
            "wdq": c(W_DQ[ci * HD:(ci + 1) * HD, :]),
            "wqr": c(W_QR[:, ci * HL * ROPE:(ci + 1) * HL * ROPE]),
            "wuk": c(W_UQ_UK[:, ci * HL * KVLR:(ci + 1) * HL * KVLR]),
            "ckv": c(ckv[:, ci * KVL:(ci + 1) * KVL, :]),
            "kpe": c(kpe[:, ci * KVL:(ci + 1) * KVL, :]),
            "cost": c(cost[ci * KVL:(ci + 1) * KVL, :]),
            "sint": c(sint[ci * KVL:(ci + 1) * KVL, :]),
            "ident": ident,
            "wuvo": c(W_UV_O[ci * HL * KVLR:(ci + 1) * HL * KVLR, :]),
        })
    return in_maps


def kernel(**inputs) -> np.ndarray:
    from concourse import bass_utils

    if "nc" not in _CACHE:
        _CACHE["nc"] = build_nc()
    nc = _CACHE["nc"]
    in_maps = make_in_maps(**inputs)
    res = bass_utils.run_bass_kernel_spmd(nc, in_maps, core_ids=list(range(N_CORES)))
    return np.asarray(res.results[0]["out"], np.float32)

